# revision 41
# baseline (speedup 1.0000x reference)
"""Trainium2 Bass kernel for nn_DeconvSlimCapsule3D (ConvTranspose3d capsule
layer with sabour dynamic routing), SPMD across 8 NeuronCores.

Sharding: core c = b*4 + s  (b = batch in {0,1}, s = D-slab in {0..3}).
Each core computes output D-planes [8s, 8s+8) of the 32^3 volume for batch b
from a 6-plane halo'd input slab. Zero inter-core communication.

v4 design: NPAR=2 -> 4 blocks of 2048 positions, with MANUAL emission-level
software pipelining: the front (deconv) and n2 of block b+1 are emitted
interleaved inside the routing iterations of block b, so the in-order
per-engine queues always have independent work (keeps PE HAM-warm and fills
DVE/ScalarE gaps). Squares on DVE (avoids ACT table thrash); rsqrt via Ln/Exp
on ScalarE (same table set as routing's Exp); 1/Z via reciprocal_approx_fast;
fp16 output.

Per-block layouts (BLK=2048 positions = 2 parities x 1024, h = par4):
  votes   [128 caps=(od*16+oa), 8j x 2048]        fp16
  preact  [128 caps, 2048]                         fp16
  compact [128 = 64h+8j+od, 1024]                  logits/el/route/R2
  chunked [32 = 16h+8c4+{od|j}, 512]               n1/Z/nn  (c = 2h+c4)
"""
import numpy as np
import ml_dtypes

B, IN_DIM, OUT_DIM, IN_ATOMS, OUT_ATOMS = 2, 8, 8, 16, 16
K, STRIDE, PAD = 4, 2, 1
CH = IN_ATOMS
D = 16
DO = 32
SLAB = 6 * 18 * 18  # 1944
F16 = np.float16

NPAR = 2               # parities per block
NBLK = 8 // NPAR       # 4 blocks
BLK = NPAR * 1024      # 2048
HALF = BLK // 2        # 1024
NCH = BLK // 512       # 4 chunks of 512

_CACHE = {}


# ---------------- host-side prep ----------------

def _k_tap(r, d):
    return 3 - 2 * d if r == 0 else 2 - 2 * d


def _build_wcol(w):
    wcol = np.zeros((8, 128, 128), np.float32)
    for p in range(8):
        rd, rh, rw = p >> 2 & 1, p >> 1 & 1, p & 1
        for t in range(8):
            dd, dh, dw = t >> 2 & 1, t >> 1 & 1, t & 1
            kk = (_k_tap(rd, dd), _k_tap(rh, dh), _k_tap(rw, dw))
            wcol[p, t * 16:t * 16 + 16, :] = w[:, :, kk[0], kk[1], kk[2]]
    return wcol.transpose(1, 0, 2).reshape(128, 8 * 128)


def _make_xrep(x, b, s):
    slab = np.zeros((IN_DIM, CH, 6, 18, 18), np.float32)
    for j0 in range(6):
        i = 4 * s - 1 + j0
        if 0 <= i < D:
            slab[:, :, j0, 1:17, 1:17] = x[b, :, :, i]
    flat = slab.reshape(IN_DIM, CH, SLAB)
    xrep = np.zeros((128, IN_DIM * SLAB), np.float32)
    for t in range(8):
        dd, dh, dw = t >> 2 & 1, t >> 1 & 1, t & 1
        off = dd * 324 + dh * 18 + dw
        n = SLAB - off
        for j in range(IN_DIM):
            xrep[t * 16:t * 16 + 16, j * SLAB:j * SLAB + n] = flat[j, :, off:]
    return xrep


def _host_constants(w, deconv_b, routing_bias):
    # oall slice s = h*8+j: caps -> 64h+8j+od   (n2 / dot reductions)
    oall = np.zeros((128, 16 * 128), np.float32)
    for h in range(2):
        for j in range(8):
            s = h * 8 + j
            for od in range(8):
                oall[od * 16:(od + 1) * 16, s * 128 + 64 * h + 8 * j + od] = 1.0
    # chunk c = 2h + c4 covers positions h*1024 + c4*512 + [0,512)
    o1c = np.zeros((128, NCH * 32), np.float32)    # caps -> 16h+8c4+od
    ozc = np.zeros((128, NCH * 32), np.float32)    # 64h+8j+od -> 16h+8c4+j
    e2c = np.zeros((32, NCH * 64), np.float32)     # n1 expand
    rzc = np.zeros((32, NCH * 64), np.float32)     # rZ expand
    gexp = np.zeros((32, NCH * 128), np.float32)   # 16h+8c4+od -> caps
    for c in range(NCH):
        h, c4 = c // 2, c % 2
        for od in range(8):
            o1c[od * 16:(od + 1) * 16, c * 32 + 16 * h + 8 * c4 + od] = 1.0
            e2c[16 * h + 8 * c4 + od, c * 64 + 8 * np.arange(8) + od] = 1.0
            gexp[16 * h + 8 * c4 + od, c * 128 + od * 16:c * 128 + (od + 1) * 16] = 1.0
        for j in range(8):
            ozc[64 * h + 8 * j:64 * h + 8 * j + 8, c * 32 + 16 * h + 8 * c4 + j] = 1.0
            rzc[16 * h + 8 * c4 + j, c * 64 + 8 * j + np.arange(8)] = 1.0
    i128 = np.eye(128, dtype=np.float32)
    rb = np.broadcast_to(routing_bias.reshape(-1), (128,)).astype(np.float32)
    bias3 = np.stack([deconv_b.astype(np.float32),
                      deconv_b.astype(np.float32) + rb, rb], axis=1)
    return {
        "wcol": _build_wcol(w).astype(F16),
        "oall": oall.astype(F16), "o1c": o1c.astype(F16),
        "ozc": ozc.astype(F16), "e2c": e2c.astype(F16), "rzc": rzc.astype(F16),
        "gexp": gexp.astype(F16), "i128": i128.astype(F16), "bias3": bias3,
    }


# ---------------- bass kernel ----------------

def _build_nc():
    import concourse.bass as bass
    import concourse.tile as tile
    from concourse import bacc, mybir
    from contextlib import ExitStack

    f32 = mybir.dt.float32
    fp16 = mybir.dt.float16
    AF = mybir.ActivationFunctionType
    ALU = mybir.AluOpType

    nc = bacc.Bacc("TRN2", target_bir_lowering=False, debug=False)

    xrep_d = nc.dram_tensor("xrep", [128, IN_DIM * SLAB], fp16, kind="ExternalInput").ap()
    xsum_d = nc.dram_tensor("xsum", [128, SLAB], fp16, kind="ExternalInput").ap()
    wcol_d = nc.dram_tensor("wcol", [128, 8 * 128], fp16, kind="ExternalInput").ap()
    oall_d = nc.dram_tensor("oall", [128, 16 * 128], fp16, kind="ExternalInput").ap()
    o1c_d = nc.dram_tensor("o1c", [128, NCH * 32], fp16, kind="ExternalInput").ap()
    ozc_d = nc.dram_tensor("ozc", [128, NCH * 32], fp16, kind="ExternalInput").ap()
    e2c_d = nc.dram_tensor("e2c", [32, NCH * 64], fp16, kind="ExternalInput").ap()
    rzc_d = nc.dram_tensor("rzc", [32, NCH * 64], fp16, kind="ExternalInput").ap()
    gexp_d = nc.dram_tensor("gexp", [32, NCH * 128], fp16, kind="ExternalInput").ap()
    i128_d = nc.dram_tensor("i128", [128, 128], fp16, kind="ExternalInput").ap()
    bias3_d = nc.dram_tensor("bias3", [128, 3], f32, kind="ExternalInput").ap()
    out_d = nc.dram_tensor("out", [128, 8 * 1024], fp16, kind="ExternalOutput").ap()

    i32 = mybir.dt.int32

    def pslice(t, p0, pn, c0, dims):
        a = t[:, :]
        return bass.AP(tensor=a.tensor, offset=a.offset + p0 * a.ap[0][0] + c0,
                       ap=[[a.ap[0][0], pn]] + dims)

    with tile.TileContext(nc) as tc, ExitStack() as ctx:
        consts = ctx.enter_context(tc.tile_pool(name="consts", bufs=1))
        xpool = ctx.enter_context(tc.tile_pool(name="xrep", bufs=1))
        vpool = ctx.enter_context(tc.tile_pool(name="votes", bufs=2))
        papool = ctx.enter_context(tc.tile_pool(name="preact", bufs=3))
        cpool = ctx.enter_context(tc.tile_pool(name="compact", bufs=2))
        bpool = ctx.enter_context(tc.tile_pool(name="big4", bufs=4))
        spool = ctx.enter_context(tc.tile_pool(name="small", bufs=2))
        opool = ctx.enter_context(tc.tile_pool(name="out", bufs=2))
        psA = ctx.enter_context(tc.tile_pool(name="psA", bufs=4, space="PSUM"))
        psB = ctx.enter_context(tc.tile_pool(name="psB", bufs=3, space="PSUM"))
        psX = ctx.enter_context(tc.tile_pool(name="psX", bufs=1, space="PSUM"))

        xrep_sb = xpool.tile([128, IN_DIM * SLAB], fp16)
        nc.sync.dma_start(xrep_sb, xrep_d)
        xsum_sb = xpool.tile([128, SLAB], fp16)
        nc.sync.dma_start(xsum_sb, xsum_d)
        wcol_sb = consts.tile([128, 8 * 128], fp16)
        nc.sync.dma_start(wcol_sb, wcol_d)
        oall_sb = consts.tile([128, 16 * 128], fp16)
        nc.sync.dma_start(oall_sb, oall_d)
        o1c_sb = consts.tile([128, NCH * 32], fp16)
        nc.sync.dma_start(o1c_sb, o1c_d)
        ozc_sb = consts.tile([128, NCH * 32], fp16)
        nc.sync.dma_start(ozc_sb, ozc_d)
        e2c_sb = consts.tile([32, NCH * 64], fp16)
        nc.sync.dma_start(e2c_sb, e2c_d)
        rzc_sb = consts.tile([32, NCH * 64], fp16)
        nc.sync.dma_start(rzc_sb, rzc_d)
        gexp_sb = consts.tile([32, NCH * 128], fp16)
        nc.sync.dma_start(gexp_sb, gexp_d)
        i128_sb = consts.tile([128, 128], fp16)
        nc.sync.dma_start(i128_sb, i128_d)
        bias_sb = consts.tile([128, 3], f32)
        nc.sync.dma_start(bias_sb, bias3_d)

        def window(src, j, p, h2):
            rd, rh, rw = p >> 2 & 1, p >> 1 & 1, p & 1
            base = (j * SLAB if j is not None else 0) + rd * 324 + rh * 18 + rw + h2 * 648
            a = src[:, :]
            return bass.AP(tensor=a.tensor, offset=a.offset + base,
                           ap=[list(a.ap[0]), [324, 2], [18, 16], [1, 16]])

        def newton_rsqrt(x_ap, out_ap, p0, pn, nm):
            # out = rsqrt(x); x_ap [pn,512] f32 (PSUM ok) at partition p0.
            t1 = spool.tile([128, 512], i32, tag="nw1", name=f"nw1{nm}")
            yy = spool.tile([128, 512], f32, tag="nw2", name=f"nw2{nm}")
            t1a = pslice(t1, p0, pn, 0, [[1, 512]])
            yya = pslice(yy, p0, pn, 0, [[1, 512]])
            xi = x_ap.bitcast(i32)
            nc.vector.tensor_scalar(t1a, xi, 1, None, op0=ALU.arith_shift_right)
            nc.vector.tensor_scalar(t1a, t1a, -1, 0x5F3759DF, op0=ALU.mult,
                                    op1=ALU.add)
            cur = pslice(t1, p0, pn, 0, [[1, 512]]).bitcast(f32)
            nc.vector.tensor_mul(yya, cur, cur)
            nc.vector.scalar_tensor_tensor(yya, yya, -0.5, x_ap,
                                           op0=ALU.mult, op1=ALU.mult)
            nc.vector.scalar_tensor_tensor(out_ap, yya, 1.5, cur,
                                           op0=ALU.add, op1=ALU.mult)

        st = {}

        def alloc_block(b):
            st[b] = {
                "votes": vpool.tile([128, 8 * BLK], fp16, tag="votes",
                                    name=f"votes{b}"),
                "preact": papool.tile([128, BLK], fp16, tag="pa", name=f"pa0_{b}"),
            }

        def front_units(b):
            votes, preact = st[b]["votes"], st[b]["preact"]
            for par4 in range(NPAR):
                p = NPAR * b + par4
                for j in range(8):
                    for h2 in (0, 1):
                        ps = psA.tile([128, 512], f32, tag="A",
                                      name=f"psf{b}_{par4}")
                        mi = nc.tensor.matmul(ps,
                                              wcol_sb[:, p * 128:(p + 1) * 128],
                                              window(xrep_sb, j, p, h2),
                                              start=True, stop=True)
                        if h2 == 1:
                            mi.ldweights = False
                        vdst = votes[:, j * BLK + par4 * 1024 + h2 * 512:
                                     j * BLK + par4 * 1024 + h2 * 512 + 512]
                        nc.scalar.activation(vdst, ps, AF.Identity,
                                             bias=bias_sb[:, 0:1])
                    yield
                for h2 in (0, 1):
                    ps = psA.tile([128, 512], f32, tag="A", name=f"psfx{b}_{par4}")
                    mi = nc.tensor.matmul(ps, wcol_sb[:, p * 128:(p + 1) * 128],
                                          window(xsum_sb, None, p, h2),
                                          start=True, stop=True)
                    if h2 == 1:
                        mi.ldweights = False
                    nc.scalar.activation(
                        preact[:, par4 * 1024 + h2 * 512:
                               par4 * 1024 + h2 * 512 + 512],
                        ps, AF.Identity, scale=0.125, bias=bias_sb[:, 1:2])
                yield

        def n2_units(b):
            votes = st[b]["votes"]
            st[b]["R2"] = cpool.tile([128, HALF], fp16, tag="R2", name=f"R2_{b}")
            R2 = st[b]["R2"]
            for c4 in range(2):
                psn2 = psA.tile([128, 512], f32, tag="A", name=f"psn2_{b}")
                sqs_ = []
                for h in range(2):
                    co = h * 1024 + c4 * 512
                    sq = bpool.tile([128, 8 * 512], fp16, tag="big4",
                                    name=f"sq{b}")
                    va = pslice(votes, 0, 128, co, [[BLK, 8], [1, 512]])
                    nc.vector.tensor_mul(
                        sq[:, :].rearrange("p (j n) -> p j n", j=8), va, va)
                    sqs_.append(sq)
                first = True
                for h in range(2):
                    sq = sqs_[h]
                    for j in range(8):
                        s = h * 8 + j
                        nc.tensor.matmul(psn2, oall_sb[:, s * 128:(s + 1) * 128],
                                         sq[:, j * 512:(j + 1) * 512],
                                         start=first, stop=(h == 1 and j == 7))
                        first = False
                newton_rsqrt(psn2[:, :], R2[:, c4 * 512:c4 * 512 + 512],
                             0, 128, f"r2_{b}_{c4}")
                yield

        def stage_a(b, it, pump):
            votes, preact = st[b]["votes"], st[b]["preact"]
            sqp = cpool.tile([128, BLK], fp16, tag="sqp", name=f"sqp{b}_{it}")
            nc.vector.tensor_mul(sqp, preact, preact)
            prs = []
            for h in range(2):
                for g in (0, 1):
                    pr = bpool.tile([128, 4 * 1024], fp16, tag="big4",
                                    name=f"pr{b}_{it}")
                    va = pslice(votes, 0, 128, (4 * g) * BLK + h * 1024,
                                [[BLK, 4], [1, 1024]])
                    pb = pslice(preact, 0, 128, h * 1024, [[0, 4], [1, 1024]])
                    nc.vector.tensor_mul(
                        pr[:, :].rearrange("p (j n) -> p j n", j=4), va, pb)
                    prs.append(pr)
            # n1 first (small PE work while DVE builds pr tiles), newton runs
            # on DVE concurrently with the 32-MM dot stream
            psn1 = psX.tile([128, 512], f32, tag="aux", name=f"psn1_{b}_{it}")
            for c in range(NCH):
                nc.tensor.matmul(pslice(psn1, 0, 32, 0, [[1, 512]]),
                                 o1c_sb[:, c * 32:(c + 1) * 32],
                                 sqp[:, c * 512:(c + 1) * 512],
                                 start=(c == 0), stop=(c == NCH - 1))
            rsq1 = spool.tile([32, 512], fp16, tag="rsq1", name=f"rsq1_{b}_{it}")
            n1a = pslice(psn1, 0, 32, 0, [[1, 512]])
            newton_rsqrt(n1a, rsq1[:, :], 0, 32, f"n1_{b}_{it}")
            pump()
            pump()
            psdots = [psB.tile([128, 512], f32, tag="B", name=f"psd{b}_{it}")
                      for _ in range(2)]
            for h in range(2):
                for g in (0, 1):
                    pr = prs[h * 2 + g]
                    for jj in range(4):
                        j = 4 * g + jj
                        for c4 in range(2):
                            rhs = pr[:, jj * 1024 + c4 * 512:
                                     jj * 1024 + c4 * 512 + 512]
                            mi = nc.tensor.matmul(
                                psdots[c4],
                                oall_sb[:, (h * 8 + j) * 128:
                                        (h * 8 + j) * 128 + 128],
                                rhs, start=(h == 0 and j == 0),
                                stop=(h == 1 and j == 7))
                            if c4 == 1:
                                mi.ldweights = False
            pump()
            return psdots, rsq1

        def stage_b(b, it, psdots, rsq1, logits, pump):
            for c4 in range(2):
                psr1e = psB.tile([128, 512], f32, tag="B", name=f"psr1e{b}_{it}")
                for h in range(2):
                    c = 2 * h + c4
                    nc.tensor.matmul(pslice(psr1e, 64 * h, 64, 0, [[1, 512]]),
                                     pslice(e2c_sb, 0, 32, c * 64, [[1, 64]]),
                                     rsq1, start=True, stop=True)
                t1 = spool.tile([128, 512], fp16, tag="t1", name=f"t1_{b}_{it}")
                R2 = st[b]["R2"]
                nc.vector.tensor_mul(t1, R2[:, c4 * 512:c4 * 512 + 512], psr1e)
                lsl = logits[:, c4 * 512:c4 * 512 + 512]
                if it == 1:
                    nc.vector.tensor_mul(lsl, t1, psdots[c4])
                else:
                    nc.vector.tensor_mul(t1, t1, psdots[c4])
                    nc.vector.tensor_add(lsl, lsl, t1)
            pump()
            el = cpool.tile([128, HALF], fp16, tag="el", name=f"el{b}_{it}")
            psz = psX.tile([128, 512], f32, tag="aux", name=f"psz{b}_{it}")
            for c4 in range(2):
                nc.scalar.activation(el[:, c4 * 512:c4 * 512 + 512],
                                     logits[:, c4 * 512:c4 * 512 + 512], AF.Exp)
                for c in (c4, 2 + c4):
                    nc.tensor.matmul(pslice(psz, 0, 32, 0, [[1, 512]]),
                                     ozc_sb[:, c * 32:(c + 1) * 32],
                                     el[:, c4 * 512:c4 * 512 + 512],
                                     start=(c4 == 0 and c == 0),
                                     stop=(c4 == 1 and c == 3))
            rzf = spool.tile([32, 512], f32, tag="rzf", name=f"rzf{b}_{it}")
            rzg = spool.tile([32, 512], f32, tag="rzg", name=f"rzg{b}_{it}")
            rz16 = spool.tile([32, 512], fp16, tag="rz16", name=f"rz16_{b}_{it}")
            za = pslice(psz, 0, 32, 0, [[1, 512]])
            nc.vector.tensor_copy(rzg, za)
            nc.vector.reciprocal_approx_fast(out=rzf, in_=rzg)
            nc.vector.tensor_copy(rz16, rzf)
            pump()
            route = cpool.tile([128, HALF], fp16, tag="route", name=f"rt{b}_{it}")
            for c4 in range(2):
                psrze = psB.tile([128, 512], f32, tag="B", name=f"psrze{b}_{it}")
                for h in range(2):
                    c = 2 * h + c4
                    nc.tensor.matmul(pslice(psrze, 64 * h, 64, 0, [[1, 512]]),
                                     pslice(rzc_sb, 0, 32, c * 64, [[1, 64]]),
                                     rz16, start=True, stop=True)
                nc.vector.tensor_mul(route[:, c4 * 512:c4 * 512 + 512],
                                     el[:, c4 * 512:c4 * 512 + 512], psrze)
            return route

        def stage_c(b, it, route, pump):
            votes = st[b]["votes"]
            preact_new = papool.tile([128, BLK], fp16, tag="pa",
                                     name=f"pa{b}_{it}")
            ra = route[:, :]
            reps = {}
            for h in range(2):
                reps[h] = (bpool.tile([128, 4 * 1024], fp16, tag="big4",
                                      name=f"repA{b}_{it}"),
                           bpool.tile([128, 4 * 1024], fp16, tag="big4",
                                      name=f"repB{b}_{it}"))
                for j in range(8):
                    src = bass.AP(tensor=ra.tensor,
                                  offset=ra.offset + (64 * h + 8 * j) * ra.ap[0][0],
                                  ap=[[ra.ap[0][0], 8], [0, 16], [1, 1024]])
                    rt = reps[h][j // 4]
                    nc.gpsimd.dma_start(rt[:, (j % 4) * 1024:(j % 4) * 1024 + 1024],
                                        src)
            for h in range(2):
                rep, rep2 = reps[h]
                for g, rt in ((0, rep), (1, rep2)):
                    va = pslice(votes, 0, 128, (4 * g) * BLK + h * 1024,
                                [[BLK, 4], [1, 1024]])
                    rv = pslice(rt, 0, 128, 0, [[1024, 4], [1, 1024]])
                    nc.vector.tensor_mul(rv, va, rv)
                psjss = [psB.tile([128, 512], f32, tag="B", name=f"psjs{b}_{it}")
                         for _ in range(2)]
                for g, rt in ((0, rep), (1, rep2)):
                    for jj in range(4):
                        for c4 in range(2):
                            mi = nc.tensor.matmul(
                                psjss[c4], i128_sb,
                                rt[:, jj * 1024 + c4 * 512:jj * 1024 + c4 * 512 + 512],
                                start=(g == 0 and jj == 0),
                                stop=(g == 1 and jj == 3))
                            if not (g == 0 and jj == 0 and c4 == 0):
                                mi.ldweights = False
                for c4 in range(2):
                    nc.scalar.activation(
                        preact_new[:, h * 1024 + c4 * 512:h * 1024 + c4 * 512 + 512],
                        psjss[c4], AF.Identity, bias=bias_sb[:, 2:3])
                pump()
            st[b]["preact"] = preact_new

        def emit_squash(b):
            preact = st[b]["preact"]
            sqs = cpool.tile([128, BLK], fp16, tag="sqp", name=f"sqs{b}")
            nc.vector.tensor_mul(sqs, preact, preact)
            psnn = psX.tile([128, 512], f32, tag="aux", name=f"psnn{b}")
            for c in range(NCH):
                nc.tensor.matmul(pslice(psnn, 0, 32, 0, [[1, 512]]),
                                 o1c_sb[:, c * 32:(c + 1) * 32],
                                 sqs[:, c * 512:(c + 1) * 512],
                                 start=(c == 0), stop=(c == NCH - 1))
            # G = sqrt(nn)/(1+nn):  s=exp(0.5 ln nn);  d=recip(1+nn);  G=s*d
            nna = pslice(psnn, 0, 32, 0, [[1, 512]])
            rsqn = spool.tile([32, 512], fp16, tag="rsq1", name=f"rsqn{b}")
            newton_rsqrt(nna, rsqn[:, :], 0, 32, f"nn_{b}")
            sg = spool.tile([32, 512], fp16, tag="sg", name=f"sg{b}")
            nc.vector.tensor_mul(sg, nna, rsqn)
            nns = spool.tile([32, 512], f32, tag="rzf", name=f"nns{b}")
            nc.vector.tensor_scalar(nns, nna, 1.0, None, op0=ALU.add)
            dd = spool.tile([32, 512], f32, tag="rzg", name=f"dd{b}")
            nc.vector.reciprocal_approx_fast(out=dd, in_=nns)
            G = spool.tile([32, 512], fp16, tag="rz16", name=f"G{b}")
            nc.vector.tensor_mul(G, sg, dd)
            for c in range(NCH):
                psg = psB.tile([128, 512], f32, tag="B", name=f"psg{b}")
                nc.tensor.matmul(psg, pslice(gexp_sb, 0, 32, c * 128, [[1, 128]]),
                                 G, start=True, stop=True)
                gS = spool.tile([128, 512], fp16, tag="gS", name=f"gS{b}")
                nc.scalar.activation(gS, psg, AF.Identity)
                outt = opool.tile([128, 512], fp16, tag="out", name=f"ot{b}")
                nc.vector.tensor_mul(outt, preact[:, c * 512:(c + 1) * 512], gS)
                nc.sync.dma_start(out_d[:, b * BLK + c * 512:b * BLK + c * 512 + 512],
                                  outt)

        # ---------------- emission (PUMPED=1 interleaves next block) ----------------
        import os as _os
        from itertools import chain as _chain
        PUMPED = _os.environ.get("KPUMP", "1") == "1"
        alloc_block(0)
        for _ in front_units(0):
            pass
        for _ in n2_units(0):
            pass
        for b in range(NBLK):
            if b + 1 < NBLK:
                alloc_block(b + 1)
                gen = _chain(front_units(b + 1), n2_units(b + 1))
            else:
                gen = iter(())

            if PUMPED:
                def pump(g=gen):
                    next(g, None)
            else:
                def pump(g=None):
                    pass

            logits = cpool.tile([128, HALF], fp16, tag="logits", name=f"lg{b}")
            for it in (1, 2):
                psdots, rsq1 = stage_a(b, it, pump)
                route = stage_b(b, it, psdots, rsq1, logits, pump)
                stage_c(b, it, route, pump)
            emit_squash(b)
            for _ in gen:
                pass

    nc.compile()
    return nc


# ---------------- public entry point ----------------

def kernel(x, w, deconv_b, routing_bias):
    from concourse.bass_utils import run_bass_kernel_spmd

    x = np.asarray(x, np.float32)
    w = np.asarray(w, np.float32)
    deconv_b = np.asarray(deconv_b, np.float32)
    routing_bias = np.asarray(routing_bias, np.float32)

    if "nc" not in _CACHE:
        _CACHE["nc"] = _build_nc()
    nc = _CACHE["nc"]

    consts = _host_constants(w, deconv_b, routing_bias)
    in_maps = []
    for c in range(8):
        b, s = c // 4, c % 4
        m = dict(consts)
        xr = _make_xrep(x, b, s)
        m["xrep"] = xr.astype(F16)
        m["xsum"] = xr.reshape(128, IN_DIM, SLAB).sum(axis=1).astype(F16)
        in_maps.append(m)

    res = run_bass_kernel_spmd(nc, in_maps, list(range(8)),
                               trace=bool(_CACHE.get("trace")),
                               tmpdir=_CACHE.get("trace_tmpdir"))
    _CACHE["last_res"] = res

    out = np.zeros((B, OUT_DIM, OUT_ATOMS, DO, DO, DO), np.float32)
    for c in range(8):
        b, s = c // 4, c % 4
        blk = np.asarray(res.results[c]["out"], np.float32)
        blk = blk.reshape(OUT_DIM, OUT_ATOMS, 2, 2, 2, 4, 16, 16)
        t = blk.transpose(0, 1, 5, 2, 6, 3, 7, 4)  # od,oa,a,rd,bh,rh,bw,rw
        out[b, :, :, 8 * s:8 * s + 8, :, :] = t.reshape(OUT_DIM, OUT_ATOMS, 8, 32, 32)
    return out


# revision 42
# speedup vs baseline: 1.0148x; 1.0148x over previous
"""Trainium2 Bass kernel for nn_DeconvSlimCapsule3D (ConvTranspose3d capsule
layer with sabour dynamic routing), SPMD across 8 NeuronCores.

Sharding: core c = b*4 + s  (b = batch in {0,1}, s = D-slab in {0..3}).
Each core computes output D-planes [8s, 8s+8) of the 32^3 volume for batch b
from a 6-plane halo'd input slab. Zero inter-core communication.

v4 design: NPAR=2 -> 4 blocks of 2048 positions, with MANUAL emission-level
software pipelining: the front (deconv) and n2 of block b+1 are emitted
interleaved inside the routing iterations of block b, so the in-order
per-engine queues always have independent work (keeps PE HAM-warm and fills
DVE/ScalarE gaps). Squares on DVE (avoids ACT table thrash); rsqrt via Ln/Exp
on ScalarE (same table set as routing's Exp); 1/Z via reciprocal_approx_fast;
fp16 output.

Per-block layouts (BLK=2048 positions = 2 parities x 1024, h = par4):
  votes   [128 caps=(od*16+oa), 8j x 2048]        fp16
  preact  [128 caps, 2048]                         fp16
  compact [128 = 64h+8j+od, 1024]                  logits/el/route/R2
  chunked [32 = 16h+8c4+{od|j}, 512]               n1/Z/nn  (c = 2h+c4)
"""
import numpy as np
import ml_dtypes

B, IN_DIM, OUT_DIM, IN_ATOMS, OUT_ATOMS = 2, 8, 8, 16, 16
K, STRIDE, PAD = 4, 2, 1
CH = IN_ATOMS
D = 16
DO = 32
SLAB = 6 * 18 * 18  # 1944
F16 = np.float16

NPAR = 2               # parities per block
NBLK = 8 // NPAR       # 4 blocks
BLK = NPAR * 1024      # 2048
HALF = BLK // 2        # 1024
NCH = BLK // 512       # 4 chunks of 512

_CACHE = {}


# ---------------- host-side prep ----------------

def _k_tap(r, d):
    return 3 - 2 * d if r == 0 else 2 - 2 * d


def _build_wcol(w):
    wcol = np.zeros((8, 128, 128), np.float32)
    for p in range(8):
        rd, rh, rw = p >> 2 & 1, p >> 1 & 1, p & 1
        for t in range(8):
            dd, dh, dw = t >> 2 & 1, t >> 1 & 1, t & 1
            kk = (_k_tap(rd, dd), _k_tap(rh, dh), _k_tap(rw, dw))
            wcol[p, t * 16:t * 16 + 16, :] = w[:, :, kk[0], kk[1], kk[2]]
    return wcol.transpose(1, 0, 2).reshape(128, 8 * 128)


def _make_xrep(x, b, s):
    slab = np.zeros((IN_DIM, CH, 6, 18, 18), np.float32)
    for j0 in range(6):
        i = 4 * s - 1 + j0
        if 0 <= i < D:
            slab[:, :, j0, 1:17, 1:17] = x[b, :, :, i]
    flat = slab.reshape(IN_DIM, CH, SLAB)
    xrep = np.zeros((128, IN_DIM * SLAB), np.float32)
    for t in range(8):
        dd, dh, dw = t >> 2 & 1, t >> 1 & 1, t & 1
        off = dd * 324 + dh * 18 + dw
        n = SLAB - off
        for j in range(IN_DIM):
            xrep[t * 16:t * 16 + 16, j * SLAB:j * SLAB + n] = flat[j, :, off:]
    return xrep


def _host_constants(w, deconv_b, routing_bias):
    # oall slice s = h*8+j: caps -> 64h+8j+od   (n2 / dot reductions)
    oall = np.zeros((128, 16 * 128), np.float32)
    for h in range(2):
        for j in range(8):
            s = h * 8 + j
            for od in range(8):
                oall[od * 16:(od + 1) * 16, s * 128 + 64 * h + 8 * j + od] = 1.0
    # chunk c = 2h + c4 covers positions h*1024 + c4*512 + [0,512)
    o1c = np.zeros((128, NCH * 32), np.float32)    # caps -> 16h+8c4+od
    ozc = np.zeros((128, NCH * 32), np.float32)    # 64h+8j+od -> 16h+8c4+j
    e2c = np.zeros((32, NCH * 64), np.float32)     # n1 expand
    rzc = np.zeros((32, NCH * 64), np.float32)     # rZ expand
    gexp = np.zeros((32, NCH * 128), np.float32)   # 16h+8c4+od -> caps
    for c in range(NCH):
        h, c4 = c // 2, c % 2
        for od in range(8):
            o1c[od * 16:(od + 1) * 16, c * 32 + 16 * h + 8 * c4 + od] = 1.0
            e2c[16 * h + 8 * c4 + od, c * 64 + 8 * np.arange(8) + od] = 1.0
            gexp[16 * h + 8 * c4 + od, c * 128 + od * 16:c * 128 + (od + 1) * 16] = 1.0
        for j in range(8):
            ozc[64 * h + 8 * j:64 * h + 8 * j + 8, c * 32 + 16 * h + 8 * c4 + j] = 1.0
            rzc[16 * h + 8 * c4 + j, c * 64 + 8 * j + np.arange(8)] = 1.0
    i128 = np.eye(128, dtype=np.float32)
    rb = np.broadcast_to(routing_bias.reshape(-1), (128,)).astype(np.float32)
    bias3 = np.stack([deconv_b.astype(np.float32),
                      deconv_b.astype(np.float32) + rb, rb], axis=1)
    return {
        "wcol": _build_wcol(w).astype(F16),
        "oall": oall.astype(F16), "o1c": o1c.astype(F16),
        "ozc": ozc.astype(F16), "e2c": e2c.astype(F16), "rzc": rzc.astype(F16),
        "gexp": gexp.astype(F16), "i128": i128.astype(F16), "bias3": bias3,
    }


# ---------------- bass kernel ----------------

def _build_nc():
    import concourse.bass as bass
    import concourse.tile as tile
    from concourse import bacc, mybir
    from contextlib import ExitStack

    f32 = mybir.dt.float32
    fp16 = mybir.dt.float16
    AF = mybir.ActivationFunctionType
    ALU = mybir.AluOpType

    nc = bacc.Bacc("TRN2", target_bir_lowering=False, debug=False)

    xrep_d = nc.dram_tensor("xrep", [128, IN_DIM * SLAB], fp16, kind="ExternalInput").ap()
    xsum_d = nc.dram_tensor("xsum", [128, SLAB], fp16, kind="ExternalInput").ap()
    wcol_d = nc.dram_tensor("wcol", [128, 8 * 128], fp16, kind="ExternalInput").ap()
    oall_d = nc.dram_tensor("oall", [128, 16 * 128], fp16, kind="ExternalInput").ap()
    o1c_d = nc.dram_tensor("o1c", [128, NCH * 32], fp16, kind="ExternalInput").ap()
    ozc_d = nc.dram_tensor("ozc", [128, NCH * 32], fp16, kind="ExternalInput").ap()
    e2c_d = nc.dram_tensor("e2c", [32, NCH * 64], fp16, kind="ExternalInput").ap()
    rzc_d = nc.dram_tensor("rzc", [32, NCH * 64], fp16, kind="ExternalInput").ap()
    gexp_d = nc.dram_tensor("gexp", [32, NCH * 128], fp16, kind="ExternalInput").ap()
    i128_d = nc.dram_tensor("i128", [128, 128], fp16, kind="ExternalInput").ap()
    bias3_d = nc.dram_tensor("bias3", [128, 3], f32, kind="ExternalInput").ap()
    out_d = nc.dram_tensor("out", [128, 8 * 1024], fp16, kind="ExternalOutput").ap()

    i32 = mybir.dt.int32

    def pslice(t, p0, pn, c0, dims):
        a = t[:, :]
        return bass.AP(tensor=a.tensor, offset=a.offset + p0 * a.ap[0][0] + c0,
                       ap=[[a.ap[0][0], pn]] + dims)

    with tile.TileContext(nc) as tc, ExitStack() as ctx:
        consts = ctx.enter_context(tc.tile_pool(name="consts", bufs=1))
        xpool = ctx.enter_context(tc.tile_pool(name="xrep", bufs=1))
        vpool = ctx.enter_context(tc.tile_pool(name="votes", bufs=2))
        papool = ctx.enter_context(tc.tile_pool(name="preact", bufs=3))
        cpool = ctx.enter_context(tc.tile_pool(name="compact", bufs=2))
        bpool = ctx.enter_context(tc.tile_pool(name="big4", bufs=4))
        spool = ctx.enter_context(tc.tile_pool(name="small", bufs=2))
        opool = ctx.enter_context(tc.tile_pool(name="out", bufs=2))
        psA = ctx.enter_context(tc.tile_pool(name="psA", bufs=4, space="PSUM"))
        psB = ctx.enter_context(tc.tile_pool(name="psB", bufs=3, space="PSUM"))
        psX = ctx.enter_context(tc.tile_pool(name="psX", bufs=1, space="PSUM"))

        xrep_sb = xpool.tile([128, IN_DIM * SLAB], fp16)
        nc.sync.dma_start(xrep_sb, xrep_d)
        xsum_sb = xpool.tile([128, SLAB], fp16)
        nc.sync.dma_start(xsum_sb, xsum_d)
        wcol_sb = consts.tile([128, 8 * 128], fp16)
        nc.sync.dma_start(wcol_sb, wcol_d)
        oall_sb = consts.tile([128, 16 * 128], fp16)
        nc.sync.dma_start(oall_sb, oall_d)
        o1c_sb = consts.tile([128, NCH * 32], fp16)
        nc.sync.dma_start(o1c_sb, o1c_d)
        ozc_sb = consts.tile([128, NCH * 32], fp16)
        nc.sync.dma_start(ozc_sb, ozc_d)
        e2c_sb = consts.tile([32, NCH * 64], fp16)
        nc.sync.dma_start(e2c_sb, e2c_d)
        rzc_sb = consts.tile([32, NCH * 64], fp16)
        nc.sync.dma_start(rzc_sb, rzc_d)
        gexp_sb = consts.tile([32, NCH * 128], fp16)
        nc.sync.dma_start(gexp_sb, gexp_d)
        i128_sb = consts.tile([128, 128], fp16)
        nc.sync.dma_start(i128_sb, i128_d)
        bias_sb = consts.tile([128, 3], f32)
        nc.sync.dma_start(bias_sb, bias3_d)

        def window(src, j, p, h2):
            rd, rh, rw = p >> 2 & 1, p >> 1 & 1, p & 1
            base = (j * SLAB if j is not None else 0) + rd * 324 + rh * 18 + rw + h2 * 648
            a = src[:, :]
            return bass.AP(tensor=a.tensor, offset=a.offset + base,
                           ap=[list(a.ap[0]), [324, 2], [18, 16], [1, 16]])

        def newton_rsqrt(x_ap, out_ap, p0, pn, nm):
            # out = rsqrt(x); x_ap [pn,512] f32 (PSUM ok) at partition p0.
            t1 = spool.tile([128, 512], i32, tag="nw1", name=f"nw1{nm}")
            yy = spool.tile([128, 512], f32, tag="nw2", name=f"nw2{nm}")
            t1a = pslice(t1, p0, pn, 0, [[1, 512]])
            yya = pslice(yy, p0, pn, 0, [[1, 512]])
            xi = x_ap.bitcast(i32)
            nc.vector.tensor_scalar(t1a, xi, 1, None, op0=ALU.arith_shift_right)
            nc.vector.tensor_scalar(t1a, t1a, -1, 0x5F3759DF, op0=ALU.mult,
                                    op1=ALU.add)
            cur = pslice(t1, p0, pn, 0, [[1, 512]]).bitcast(f32)
            nc.vector.tensor_mul(yya, cur, cur)
            nc.vector.scalar_tensor_tensor(yya, yya, -0.5, x_ap,
                                           op0=ALU.mult, op1=ALU.mult)
            nc.vector.scalar_tensor_tensor(out_ap, yya, 1.5, cur,
                                           op0=ALU.add, op1=ALU.mult)

        st = {}

        def alloc_block(b):
            st[b] = {
                "votes": vpool.tile([128, 8 * BLK], fp16, tag="votes",
                                    name=f"votes{b}"),
                "preact": papool.tile([128, BLK], fp16, tag="pa", name=f"pa0_{b}"),
            }

        def front_units(b):
            votes, preact = st[b]["votes"], st[b]["preact"]
            for par4 in range(NPAR):
                p = NPAR * b + par4
                for j in range(8):
                    for h2 in (0, 1):
                        ps = psA.tile([128, 512], f32, tag="A",
                                      name=f"psf{b}_{par4}")
                        mi = nc.tensor.matmul(ps,
                                              wcol_sb[:, p * 128:(p + 1) * 128],
                                              window(xrep_sb, j, p, h2),
                                              start=True, stop=True)
                        if h2 == 1:
                            mi.ldweights = False
                        vdst = votes[:, j * BLK + par4 * 1024 + h2 * 512:
                                     j * BLK + par4 * 1024 + h2 * 512 + 512]
                        nc.scalar.activation(vdst, ps, AF.Identity,
                                             bias=bias_sb[:, 0:1])
                    yield
                for h2 in (0, 1):
                    ps = psA.tile([128, 512], f32, tag="A", name=f"psfx{b}_{par4}")
                    mi = nc.tensor.matmul(ps, wcol_sb[:, p * 128:(p + 1) * 128],
                                          window(xsum_sb, None, p, h2),
                                          start=True, stop=True)
                    if h2 == 1:
                        mi.ldweights = False
                    nc.scalar.activation(
                        preact[:, par4 * 1024 + h2 * 512:
                               par4 * 1024 + h2 * 512 + 512],
                        ps, AF.Identity, scale=0.125, bias=bias_sb[:, 1:2])
                yield

        def n2_units(b):
            votes = st[b]["votes"]
            st[b]["R2"] = cpool.tile([128, HALF], fp16, tag="R2", name=f"R2_{b}")
            R2 = st[b]["R2"]
            for c4 in range(2):
                psn2 = psA.tile([128, 512], f32, tag="A", name=f"psn2_{b}")
                sqs_ = []
                for h in range(2):
                    co = h * 1024 + c4 * 512
                    sq = bpool.tile([128, 8 * 512], fp16, tag="big4",
                                    name=f"sq{b}")
                    va = pslice(votes, 0, 128, co, [[BLK, 8], [1, 512]])
                    nc.vector.tensor_mul(
                        sq[:, :].rearrange("p (j n) -> p j n", j=8), va, va)
                    sqs_.append(sq)
                first = True
                for h in range(2):
                    sq = sqs_[h]
                    for j in range(8):
                        s = h * 8 + j
                        nc.tensor.matmul(psn2, oall_sb[:, s * 128:(s + 1) * 128],
                                         sq[:, j * 512:(j + 1) * 512],
                                         start=first, stop=(h == 1 and j == 7))
                        first = False
                newton_rsqrt(psn2[:, :], R2[:, c4 * 512:c4 * 512 + 512],
                             0, 128, f"r2_{b}_{c4}")
                yield

        def stage_a(b, it, pump):
            votes, preact = st[b]["votes"], st[b]["preact"]
            sqp = cpool.tile([128, BLK], fp16, tag="sqp", name=f"sqp{b}_{it}")
            nc.vector.tensor_mul(sqp, preact, preact)
            prs = []
            for h in range(2):
                for g in (0, 1):
                    pr = bpool.tile([128, 4 * 1024], fp16, tag="big4",
                                    name=f"pr{b}_{it}")
                    va = pslice(votes, 0, 128, (4 * g) * BLK + h * 1024,
                                [[BLK, 4], [1, 1024]])
                    pb = pslice(preact, 0, 128, h * 1024, [[0, 4], [1, 1024]])
                    nc.vector.tensor_mul(
                        pr[:, :].rearrange("p (j n) -> p j n", j=4), va, pb)
                    prs.append(pr)
            # n1 first (small PE work while DVE builds pr tiles), newton runs
            # on DVE concurrently with the 32-MM dot stream
            psn1 = psX.tile([128, 512], f32, tag="aux", name=f"psn1_{b}_{it}")
            for c in range(NCH):
                nc.tensor.matmul(pslice(psn1, 0, 32, 0, [[1, 512]]),
                                 o1c_sb[:, c * 32:(c + 1) * 32],
                                 sqp[:, c * 512:(c + 1) * 512],
                                 start=(c == 0), stop=(c == NCH - 1))
            rsq1 = spool.tile([32, 512], fp16, tag="rsq1", name=f"rsq1_{b}_{it}")
            n1a = pslice(psn1, 0, 32, 0, [[1, 512]])
            newton_rsqrt(n1a, rsq1[:, :], 0, 32, f"n1_{b}_{it}")
            pump()
            pump()
            psdots = [psB.tile([128, 512], f32, tag="B", name=f"psd{b}_{it}")
                      for _ in range(2)]
            for h in range(2):
                for g in (0, 1):
                    pr = prs[h * 2 + g]
                    for jj in range(4):
                        j = 4 * g + jj
                        for c4 in range(2):
                            rhs = pr[:, jj * 1024 + c4 * 512:
                                     jj * 1024 + c4 * 512 + 512]
                            mi = nc.tensor.matmul(
                                psdots[c4],
                                oall_sb[:, (h * 8 + j) * 128:
                                        (h * 8 + j) * 128 + 128],
                                rhs, start=(h == 0 and j == 0),
                                stop=(h == 1 and j == 7))
                            if c4 == 1:
                                mi.ldweights = False
            pump()
            return psdots, rsq1

        def stage_b(b, it, psdots, rsq1, logits, pump):
            for c4 in range(2):
                psr1e = psB.tile([128, 512], f32, tag="B", name=f"psr1e{b}_{it}")
                for h in range(2):
                    c = 2 * h + c4
                    nc.tensor.matmul(pslice(psr1e, 64 * h, 64, 0, [[1, 512]]),
                                     pslice(e2c_sb, 0, 32, c * 64, [[1, 64]]),
                                     rsq1, start=True, stop=True)
                t1 = spool.tile([128, 512], fp16, tag="t1", name=f"t1_{b}_{it}")
                R2 = st[b]["R2"]
                nc.vector.tensor_mul(t1, R2[:, c4 * 512:c4 * 512 + 512], psr1e)
                lsl = logits[:, c4 * 512:c4 * 512 + 512]
                if it == 1:
                    nc.vector.tensor_mul(lsl, t1, psdots[c4])
                else:
                    nc.vector.tensor_mul(t1, t1, psdots[c4])
                    nc.vector.tensor_add(lsl, lsl, t1)
            pump()
            el = cpool.tile([128, HALF], fp16, tag="el", name=f"el{b}_{it}")
            psz = psX.tile([128, 512], f32, tag="aux", name=f"psz{b}_{it}")
            for c4 in range(2):
                nc.scalar.activation(el[:, c4 * 512:c4 * 512 + 512],
                                     logits[:, c4 * 512:c4 * 512 + 512], AF.Exp)
                for c in (c4, 2 + c4):
                    nc.tensor.matmul(pslice(psz, 0, 32, 0, [[1, 512]]),
                                     ozc_sb[:, c * 32:(c + 1) * 32],
                                     el[:, c4 * 512:c4 * 512 + 512],
                                     start=(c4 == 0 and c == 0),
                                     stop=(c4 == 1 and c == 3))
            rzf = spool.tile([32, 512], f32, tag="rzf", name=f"rzf{b}_{it}")
            rzg = spool.tile([32, 512], f32, tag="rzg", name=f"rzg{b}_{it}")
            rz16 = spool.tile([32, 512], fp16, tag="rz16", name=f"rz16_{b}_{it}")
            za = pslice(psz, 0, 32, 0, [[1, 512]])
            nc.vector.tensor_copy(rzg, za)
            nc.vector.reciprocal_approx_fast(out=rzf, in_=rzg)
            nc.vector.tensor_copy(rz16, rzf)
            pump()
            route = cpool.tile([128, HALF], fp16, tag="route", name=f"rt{b}_{it}")
            for c4 in range(2):
                psrze = psB.tile([128, 512], f32, tag="B", name=f"psrze{b}_{it}")
                for h in range(2):
                    c = 2 * h + c4
                    nc.tensor.matmul(pslice(psrze, 64 * h, 64, 0, [[1, 512]]),
                                     pslice(rzc_sb, 0, 32, c * 64, [[1, 64]]),
                                     rz16, start=True, stop=True)
                nc.vector.tensor_mul(route[:, c4 * 512:c4 * 512 + 512],
                                     el[:, c4 * 512:c4 * 512 + 512], psrze)
            return route

        def stage_c(b, it, route, pump):
            votes = st[b]["votes"]
            preact_new = papool.tile([128, BLK], fp16, tag="pa",
                                     name=f"pa{b}_{it}")
            ra = route[:, :]
            reps = {}
            for h in range(2):
                reps[h] = (bpool.tile([128, 4 * 1024], fp16, tag="big4",
                                      name=f"repA{b}_{it}"),
                           bpool.tile([128, 4 * 1024], fp16, tag="big4",
                                      name=f"repB{b}_{it}"))
                for j in range(8):
                    src = bass.AP(tensor=ra.tensor,
                                  offset=ra.offset + (64 * h + 8 * j) * ra.ap[0][0],
                                  ap=[[ra.ap[0][0], 8], [0, 16], [1, 1024]])
                    rt = reps[h][j // 4]
                    nc.gpsimd.dma_start(rt[:, (j % 4) * 1024:(j % 4) * 1024 + 1024],
                                        src)
            for h in range(2):
                rep, rep2 = reps[h]
                for g, rt in ((0, rep), (1, rep2)):
                    va = pslice(votes, 0, 128, (4 * g) * BLK + h * 1024,
                                [[BLK, 4], [1, 1024]])
                    rv = pslice(rt, 0, 128, 0, [[1024, 4], [1, 1024]])
                    nc.vector.tensor_mul(rv, va, rv)
                psjss = [psB.tile([128, 512], f32, tag="B", name=f"psjs{b}_{it}")
                         for _ in range(2)]
                for g, rt in ((0, rep), (1, rep2)):
                    for jj in range(4):
                        for c4 in range(2):
                            mi = nc.tensor.matmul(
                                psjss[c4], i128_sb,
                                rt[:, jj * 1024 + c4 * 512:jj * 1024 + c4 * 512 + 512],
                                start=(g == 0 and jj == 0),
                                stop=(g == 1 and jj == 3))
                            if not (g == 0 and jj == 0 and c4 == 0):
                                mi.ldweights = False
                for c4 in range(2):
                    nc.scalar.activation(
                        preact_new[:, h * 1024 + c4 * 512:h * 1024 + c4 * 512 + 512],
                        psjss[c4], AF.Identity, bias=bias_sb[:, 2:3])
                pump()
            st[b]["preact"] = preact_new

        def emit_squash(b):
            preact = st[b]["preact"]
            sqs = cpool.tile([128, BLK], fp16, tag="sqp", name=f"sqs{b}")
            nc.vector.tensor_mul(sqs, preact, preact)
            psnn = psX.tile([128, 512], f32, tag="aux", name=f"psnn{b}")
            for c in range(NCH):
                nc.tensor.matmul(pslice(psnn, 0, 32, 0, [[1, 512]]),
                                 o1c_sb[:, c * 32:(c + 1) * 32],
                                 sqs[:, c * 512:(c + 1) * 512],
                                 start=(c == 0), stop=(c == NCH - 1))
            # G = sqrt(nn)/(1+nn):  s=exp(0.5 ln nn);  d=recip(1+nn);  G=s*d
            nna = pslice(psnn, 0, 32, 0, [[1, 512]])
            rsqn = spool.tile([32, 512], fp16, tag="rsq1", name=f"rsqn{b}")
            newton_rsqrt(nna, rsqn[:, :], 0, 32, f"nn_{b}")
            sg = spool.tile([32, 512], fp16, tag="sg", name=f"sg{b}")
            nc.vector.tensor_mul(sg, nna, rsqn)
            nns = spool.tile([32, 512], f32, tag="rzf", name=f"nns{b}")
            nc.vector.tensor_scalar(nns, nna, 1.0, None, op0=ALU.add)
            dd = spool.tile([32, 512], f32, tag="rzg", name=f"dd{b}")
            nc.vector.reciprocal_approx_fast(out=dd, in_=nns)
            G = spool.tile([32, 512], fp16, tag="rz16", name=f"G{b}")
            nc.vector.tensor_mul(G, sg, dd)
            for c in range(NCH):
                psg = psB.tile([128, 512], f32, tag="B", name=f"psg{b}")
                nc.tensor.matmul(psg, pslice(gexp_sb, 0, 32, c * 128, [[1, 128]]),
                                 G, start=True, stop=True)
                outt = opool.tile([128, 512], fp16, tag="out", name=f"ot{b}")
                nc.vector.tensor_mul(outt, preact[:, c * 512:(c + 1) * 512], psg)
                nc.sync.dma_start(out_d[:, b * BLK + c * 512:b * BLK + c * 512 + 512],
                                  outt)

        # ---------------- emission (PUMPED=1 interleaves next block) ----------------
        import os as _os
        from itertools import chain as _chain
        PUMPED = _os.environ.get("KPUMP", "1") == "1"
        alloc_block(0)
        for _ in front_units(0):
            pass
        for _ in n2_units(0):
            pass
        for b in range(NBLK):
            if b + 1 < NBLK:
                alloc_block(b + 1)
                gen = _chain(front_units(b + 1), n2_units(b + 1))
            else:
                gen = iter(())

            if PUMPED:
                def pump(g=gen):
                    next(g, None)
            else:
                def pump(g=None):
                    pass

            logits = cpool.tile([128, HALF], fp16, tag="logits", name=f"lg{b}")
            for it in (1, 2):
                psdots, rsq1 = stage_a(b, it, pump)
                route = stage_b(b, it, psdots, rsq1, logits, pump)
                stage_c(b, it, route, pump)
            emit_squash(b)
            for _ in gen:
                pass

    nc.compile()
    return nc


# ---------------- public entry point ----------------

def kernel(x, w, deconv_b, routing_bias):
    from concourse.bass_utils import run_bass_kernel_spmd

    x = np.asarray(x, np.float32)
    w = np.asarray(w, np.float32)
    deconv_b = np.asarray(deconv_b, np.float32)
    routing_bias = np.asarray(routing_bias, np.float32)

    if "nc" not in _CACHE:
        _CACHE["nc"] = _build_nc()
    nc = _CACHE["nc"]

    consts = _host_constants(w, deconv_b, routing_bias)
    in_maps = []
    for c in range(8):
        b, s = c // 4, c % 4
        m = dict(consts)
        xr = _make_xrep(x, b, s)
        m["xrep"] = xr.astype(F16)
        m["xsum"] = xr.reshape(128, IN_DIM, SLAB).sum(axis=1).astype(F16)
        in_maps.append(m)

    res = run_bass_kernel_spmd(nc, in_maps, list(range(8)),
                               trace=bool(_CACHE.get("trace")),
                               tmpdir=_CACHE.get("trace_tmpdir"))
    _CACHE["last_res"] = res

    out = np.zeros((B, OUT_DIM, OUT_ATOMS, DO, DO, DO), np.float32)
    for c in range(8):
        b, s = c // 4, c % 4
        blk = np.asarray(res.results[c]["out"], np.float32)
        blk = blk.reshape(OUT_DIM, OUT_ATOMS, 2, 2, 2, 4, 16, 16)
        t = blk.transpose(0, 1, 5, 2, 6, 3, 7, 4)  # od,oa,a,rd,bh,rh,bw,rw
        out[b, :, :, 8 * s:8 * s + 8, :, :] = t.reshape(OUT_DIM, OUT_ATOMS, 8, 32, 32)
    return out
